# revision 32
# baseline (speedup 1.0000x reference)
"""AcceleratedInnerShiftTriple kernel for 8 TRN2 NeuronCores.

Reference math (B=4, C=512, H=W=64, N=4096, C2=256):
  former, latter = x[:, :256], x[:, 256:]   (each (B, 256, N) after reshape)
  flag[n] = mask[n] >= 1
  cos[b,n,m] = <latter_n/|latter_n|, latter_m/|latter_m|>, masked candidates m
  excluded (-inf); nn = argmax_m; shift = former[:, :, nn] where flag else 0
  out = concat([former, latter, shift], channel) -> (B, 768, 64, 64)

Reductions used:
  * out[:, :512] == x verbatim; only `shift` needs computing.
  * Only masked positions are queries, only unmasked positions candidates;
    both compacted host-side from the runtime mask (order-preserving).
  * Device = coarse-scan ranking accelerator (classic ANN retrieval):
    bf16 cosine matmul on the PE (f32 PSUM accumulate) + one segmented
    reduce_max pass on the vector engine per score chunk -> per-query
    128-wide segment maxima. Host picks every segment within MARGIN of the
    global max (MARGIN >> 2*bf16-score-error, measured 1.6e-3 bound vs
    8e-3 margin) and rescores only those candidates in float64 -> the
    argmax is exact, bit-identical gather from `former`.

Sharding: 2 cores per batch element, each takes half the masked queries:
  512 queries x 3072 candidates x K=256 per core.
"""

import numpy as np

EPS = 1e-8
P = 128
CHUNK = 1024  # candidate chunk width (2 PSUM banks)
SEG = 128     # segment width for the device segmented max
MARGIN = 8e-3
NEG = -1e30

# test.py toggles these for profiling
TRACE = False
LAST_EXEC_NS = None
LAST_RESULTS = None
LAST_TRACE = None
LAST_PROFILE_JSON = None


def _install_profiling():
    """Register the NTFF profile hook that this container's antenv lacks.

    Best-effort: profiling is test-only; kernel correctness never depends
    on it.
    """
    import sys
    import types

    try:
        from antenv.axon_hooks import get_axon_ntff_profile_hook  # noqa: F401

        return True
    except ImportError:
        pass
    try:
        import antenv
        from trn_agent_boot.trn_boot import _ntff_profile_via_ctypes

        mod = types.ModuleType("antenv.axon_hooks")
        state = {}
        mod.set_axon_ntff_profile_hook = lambda h: state.update(hook=h)
        mod.get_axon_ntff_profile_hook = lambda: state.get("hook")
        sys.modules["antenv.axon_hooks"] = mod
        antenv.axon_hooks = mod
        mod.set_axon_ntff_profile_hook(
            _ntff_profile_via_ctypes("/opt/axon/libaxon_pjrt.so")
        )
        from concourse import bass_utils

        bass_utils.upload_artifacts = lambda tmpdir: tmpdir  # no S3 here
        return True
    except Exception as e:  # pragma: no cover
        print(f"profiling hook install failed: {e}")
        return False


def _build(nqp, ncp, kdim, ncand=None):
    """SPMD graph for one core: nqp queries x ncp candidates, bf16 inputs.

    Output: per-query max score of each SEG-wide candidate segment (f32).
    ncand: number of real (non-pad) candidates; pad scores forced to NEG.
    """
    import concourse.mybir as mybir
    import concourse.tile as tile_mod
    from concourse.bacc import Bacc
    from concourse.tile import TileContext

    class FastExitTileContext(TileContext):
        """TileContext whose exit skips the device-side semaphore clear and
        second all-engine barrier: every NEFF execution re-clears the kernel
        semaphore range in its own preamble, so for a single-TileContext
        kernel the tail clear only costs time."""

        def _drain_and_barrier(self, tick_clock, wait_clock):
            drain_inst = self.nc.sync.drain()
            wait_clock.add_sem_waits(
                drain_inst.ins,
                tile_mod.ScopedClock({None: tick_clock.global_clock}),
            )
            self.nc.all_engine_barrier()
            popped = self.nc._tile_sem_poison_stack.pop()
            assert popped is self._sem_poison
            sems = list(self.sems.allocated().values())
            sem_nums = [s.num if hasattr(s, "num") else s for s in sems]
            self.nc._state.prepend_free_semaphores(sem_nums)
            for poison_set in self.nc._tile_sem_poison_stack:
                poison_set.update(sem_nums)

    f32 = mybir.dt.float32
    bf16 = mybir.dt.bfloat16

    assert nqp % P == 0 and ncp % CHUNK == 0 and kdim == 256
    nqb = nqp // P
    nch = ncp // CHUNK
    spc = CHUNK // SEG  # segments per chunk
    nseg = ncp // SEG
    if ncand is None:
        ncand = ncp

    nc = Bacc()
    w = nqp + ncp
    qc_ext = nc.declare_dram_parameter("qc", [P, 2, w], bf16, isOutput=False)
    seg_ext = nc.declare_dram_parameter("segmax", [nqp, nseg], f32, isOutput=True)

    with FastExitTileContext(nc) as tc:
        with (
            tc.tile_pool(name="persist", bufs=1) as persist,
            tc.tile_pool(name="psum", bufs=4, space="PSUM") as psum_pool,
        ):
            # split loads finely: per-kc query tiles + 512-wide candidate
            # tiles so the first matmuls start as soon as a slice lands and
            # the DMAs spread over parallel queues
            q_sb = []
            for kc in range(2):
                t = persist.tile([P, nqp], bf16, tag=f"q{kc}")
                nc.sync.dma_start(out=t[:], in_=qc_ext[:, kc, 0:nqp])
                q_sb.append(t)
            nsub = ncp // 512
            c_sb = []
            for sb in range(nsub):
                lo = nqp + sb * 512
                t = persist.tile([P, 2, 512], bf16, tag=f"c{sb}")
                nc.sync.dma_start(out=t[:], in_=qc_ext[:, :, lo : lo + 512])
                c_sb.append(t)

            sm_all = persist.tile([P, nqb, nseg], f32)
            for qb in range(nqb):
                for ch in range(nch):
                    ps = psum_pool.tile([P, CHUNK], f32, tag="ps")
                    for sub in range(0, CHUNK, 512):
                        for kc in range(2):
                            nc.tensor.matmul(
                                out=ps[:, sub : sub + 512],
                                lhsT=q_sb[kc][:, qb * P : (qb + 1) * P],
                                rhs=c_sb[(ch * CHUNK + sub) // 512][:, kc, :],
                                start=(kc == 0),
                                stop=(kc == 1),
                            )
                    if ncand < (ch + 1) * CHUNK:
                        pad0 = max(0, ncand - ch * CHUNK)
                        nc.vector.memset(ps[:, pad0:], NEG)
                    nc.vector.tensor_reduce(
                        out=sm_all[:, qb, ch * spc : (ch + 1) * spc],
                        in_=ps[:].rearrange("p (s e) -> p s e", e=SEG),
                        axis=mybir.AxisListType.X,
                        op=mybir.AluOpType.max,
                    )
            nc.sync.dma_start(
                out=seg_ext[:].rearrange("(a p) b -> p a b", p=P),
                in_=sm_all[:],
            )
    if not nc.is_finalized():
        nc.finalize()
    return nc


def _ceil_to(v, m):
    return max(m, ((v + m - 1) // m) * m)


def kernel(x, mask):
    global LAST_EXEC_NS, LAST_RESULTS
    x = np.ascontiguousarray(np.asarray(x, dtype=np.float32))
    mask = np.asarray(mask, dtype=np.float32)
    B, C, H, W = x.shape
    C2 = C // 2
    N = H * W
    former = x[:, :C2].reshape(B, C2, N)
    latter = x[:, C2:].reshape(B, C2, N)
    flag = mask.reshape(N) >= 1.0
    qs = np.flatnonzero(flag)
    cs = np.flatnonzero(~flag)
    nq, ncand = len(qs), len(cs)

    shift = np.zeros((B, C2, N), np.float32)
    if nq > 0 and ncand == 0:
        # all candidates masked: argmax of all -inf rows is 0
        shift[:, :, qs] = former[:, :, 0][:, :, None]
    elif nq > 0:
        import ml_dtypes

        assert B * 2 == 8, "sharding hardcoded for B=4 over 8 cores"
        h = (nq + 1) // 2
        halves = [qs[:h], qs[h:]]
        nqp = _ceil_to(h, P)
        ncp = _ceil_to(ncand, CHUNK)
        nseg = ncp // SEG

        # normalize BOTH sides (query scale never changes the argmax, but
        # bounding scores to cosines makes the bf16 error margin data-
        # scale-independent)
        qn = latter[:, :, qs] / (
            np.linalg.norm(latter[:, :, qs], axis=1, keepdims=True) + EPS
        )
        nrm = np.linalg.norm(latter[:, :, cs], axis=1)
        cn = latter[:, :, cs] * (1.0 / (nrm + EPS))[:, None, :]

        in_maps = []
        for core in range(8):
            b, hi = divmod(core, 2)
            lo = hi * h
            qh = halves[hi]
            qc = np.zeros((P, 2, nqp + ncp), ml_dtypes.bfloat16)
            if len(qh):
                qc[:, :, : len(qh)] = (
                    qn[b][:, lo : lo + len(qh)]
                    .reshape(2, P, len(qh))
                    .transpose(1, 0, 2)
                    .astype(ml_dtypes.bfloat16)
                )
            qc[:, :, nqp : nqp + ncand] = (
                cn[b].reshape(2, P, ncand).transpose(1, 0, 2)
                .astype(ml_dtypes.bfloat16)
            )
            in_maps.append({"qc": qc})

        from concourse.bass_utils import run_bass_kernel_spmd

        trace = TRACE and _install_profiling()
        nc = _build(nqp, ncp, C2, ncand=ncand)
        res = run_bass_kernel_spmd(nc, in_maps, core_ids=list(range(8)), trace=trace)
        LAST_EXEC_NS = res.exec_time_ns
        LAST_RESULTS = res.results
        global LAST_TRACE, LAST_PROFILE_JSON
        if res.instructions_and_trace is not None:
            LAST_TRACE = res.instructions_and_trace[1]
        LAST_PROFILE_JSON = res.profile_json

        cn64 = cn.astype(np.float64)
        for core in range(8):
            b, hi = divmod(core, 2)
            qh = halves[hi]
            if not len(qh):
                continue
            sm = res.results[core]["segmax"][: len(qh)]  # (nqh, nseg) f32
            gm = sm.max(axis=1)
            pick = sm >= (gm[:, None] - MARGIN)  # (nqh, nseg)
            # exact float64 rescore of every candidate in a picked segment;
            # MARGIN >> 2*max|bf16 - f64| guarantees the true winner's
            # segment is picked
            qrow, srow = np.nonzero(pick)
            lo = hi * h
            latq64 = qn[b][:, lo : lo + len(qh)].astype(np.float64)
            win = np.full(len(qh), -1, np.int64)
            best = np.full(len(qh), -np.inf)
            # group by segment for contiguous gemm slices
            order = np.argsort(srow, kind="stable")
            qrow, srow = qrow[order], srow[order]
            bounds = np.searchsorted(srow, np.arange(nseg + 1))
            for s in range(nseg):
                a, e = bounds[s], bounds[s + 1]
                if a == e:
                    continue
                c0 = s * SEG
                c1 = min(c0 + SEG, ncand)
                if c1 <= c0:
                    continue
                qq = qrow[a:e]
                blockw = c1 - c0
                sc = cn64[b][:, c0:c1].T @ latq64[:, qq]  # (blockw, nq_pick)
                bi = np.argmax(sc, axis=0)
                bv = sc[bi, np.arange(len(qq))]
                # exact ties inside the segment: argmax already returns the
                # first (lowest candidate index); across segments resolved
                # below with index tiebreak
                cidx = c0 + bi
                for j, q_ in enumerate(qq):
                    v, ci = bv[j], cidx[j]
                    if v > best[q_] or (v == best[q_] and ci < win[q_]):
                        best[q_] = v
                        win[q_] = ci
            assert (win >= 0).all(), "segment pick missed every candidate"
            shift[b][:, qh] = former[b][:, cs].T[win].T

    out = np.concatenate([former, latter, shift], axis=1)
    return out.reshape(B, 3 * C2, H, W)
